# revision 14
# baseline (speedup 1.0000x reference)
"""Trainium2 Bass kernel for nn_Attention_43542378447097 (v3).

GroupNorm -> multi-head causal self-attention -> out-proj, then the
reference's broadcast add:

    out(B,S,C) + residual(B,C,1,C)  ->  (B,C,S,C)

    result[i, j, k, l] = A[j, k, l] + xn[i, j, l]

where A[j] = attention output (no bo) of batch j and xn[i] = groupnorm
output of batch i (+ beta + bo_eff folded in).  Output (96,96,96,96),
fp16 on device = 21.2MB/core across 8 j-sharded cores -> the out-DMA
(~71us on the scalar HWDGE ring) is the intended critical path.

v3 vs v2 (measured on HW via probe phases):
  * Assembly TT reshaped to j-quad innermost [l, i24, k96, j4]: DVE
    hits ~2.2 elem/cyc/partition (54.7us total) vs 1.2 for j-pairs
    (96us).  aarr is [l, k, j] and xarr [l, i, j] so both operands
    stream innermost step-1 quads; broadcasts sit on mid dims only.
  * Out-DMAs issued on nc.scalar (qActDynamicHW ring): 71.2us vs
    84.6us on nc.sync for the same 12 x 1.77MB transfers.
  * Attention restructured around xallT [c, b, s] (host-transposed):
    groupnorm scale folds into the contraction as xsc98 = xallT *
    scale2 with two extra matmul rows (shift-row, ones-row) carrying
    the shift*Wsum and bias terms, so q/k/v need no per-batch
    transposes or ACT scale passes.  q/k are batched over all 12
    local batches per head (N=384 matmuls).
  * Softmax: denominators via ones-matmul (replicated across
    partitions), reciprocal via the custom DVE op (~1 cyc/elem, f16
    out), and the normalize multiply runs on the otherwise-idle
    GPSIMD engine from SBUF (ACT evicts raw oT).
  * ACT only ever loads the exp_and_others table (Copy/Identity/Exp/
    Square); rsqrt stays the all-DVE Newton chain.
"""

import sys

sys.path.insert(0, "/opt/trn_rl_repo")

import numpy as np

B_TOTAL = 96
C = 96
S = 96
NH = 8
DK = 96
G = 8
NCORES = 8
JPC = B_TOTAL // NCORES  # 12 local j's (attention batches) per core
JW = 4  # assembly quad width (j's per asm group)
NJG = JPC // JW  # 3 quad groups
IW = 96 // JW  # 24 i's per asm op
EPS = 1e-5

_PROG = None


def _build_program(loop_n=1, phases="123", aspl=None, dma_ring="scalar"):
    import contextlib

    import concourse.bass as bass
    import concourse.tile as tile
    from concourse import bacc, mybir
    from concourse.dve_ops import RECIP_APPROX_FAST_CONSTS, RECIPROCAL_APPROX_FAST

    f32 = mybir.dt.float32
    f16 = mybir.dt.float16
    i32 = mybir.dt.int32
    AF = mybir.ActivationFunctionType
    ALU = mybir.AluOpType
    AX = mybir.AxisListType

    nc = bacc.Bacc(
        "TRN2",
        target_bir_lowering=False,
        debug=False,
        enable_asserts=False,
        num_devices=NCORES,
    )

    xallT_d = nc.declare_dram_parameter("xallT", [C, JPC, S], f16, isOutput=False)
    xg_d = nc.declare_dram_parameter("xg", [B_TOTAL, JPC * C], f16, isOutput=False)
    cp16_d = nc.declare_dram_parameter("cpack16", [98, 3936], f16, isOutput=False)
    cp32_d = nc.declare_dram_parameter("cpack32", [98, 216], f32, isOutput=False)
    out_d = nc.declare_dram_parameter(
        "out", [NJG, JW, C, IW, S, JW], f16, isOutput=True
    )

    with tile.TileContext(nc) as tc:
        with (
            tc.tile_pool(name="const", bufs=1) as cpool,
            tc.tile_pool(name="work", bufs=2) as work,
            tc.tile_pool(name="psum", bufs=8, space="PSUM") as pp,
        ):
            # ---- constants (packed tiles; views below) ----
            cp16 = cpool.tile([98, 3936], f16, name="cp16")
            cp32 = cpool.tile([98, 216], f32, name="cp32")
            aarr = cpool.tile([C, S, JPC], f16, name="aarr")  # [l, k, j]
            xarr = cpool.tile([C, B_TOTAL, JPC], f16, name="xarr")  # [l, i, j]
            xsc98 = cpool.tile([98, JPC, S], f16, name="xsc98")
            qkT = cpool.tile([DK, 2, NH, JPC, S], f16, name="qkT")

            wq_sb = cp16[:, 0:768].rearrange("p (h d) -> p h d", h=NH)
            wk_sb = cp16[:, 768:1536].rearrange("p (h d) -> p h d", h=NH)
            wv_sb = cp16[0:97, 1536:2304]
            wo_sb = cp16[0:DK, 2304:3072].rearrange("p (h l) -> p h l", h=NH)
            maskb_sb = cp16[0:S, 3072:3168]
            ones_sb = cp16[0:S, 3168:3264]
            iden4_sb = cp16[0:C, 3264:3648]
            iden_sb = cp16[0:C, 3648:3744]
            ones98_sb = cp16[0:C, 3744:3842]
            gvec_sb = cp32[0:C, 0:12]
            bb_sb = cp32[0:C, 12:24]
            gamma_rep = cp32[:, 24:120]
            beta_rep = cp32[:, 120:216]

            nc.sync.dma_start(out=cp16, in_=cp16_d[:])
            nc.sync.dma_start(out=cp32, in_=cp32_d[:])
            # rows 96-97 = 1.0 once; the loop overwrites row 96 with the
            # shift row each iteration (engines need 32-aligned start
            # partitions, so the two rows are set together)
            nc.vector.memset(xsc98[96:98, :, :], 1.0)

            inv_na = 1.0 / (C * C // G)  # 1/1152 per (batch, group)
            rc = RECIP_APPROX_FAST_CONSTS

            loop_cm = (
                tc.For_i(0, loop_n, 1) if loop_n > 1 else contextlib.nullcontext()
            )
            loop_cm.__enter__()

            def newton_rsqrt(veps, tag, iters=2):
                """rstd = rsqrt(veps), all-DVE (quake seed + Newton steps)
                so ACT only ever needs the Exp table set."""
                shp = list(veps.shape)
                iv = veps.bitcast(i32)
                ineg = work.tile(shp, i32, tag="sti", bufs=8, name="ineg" + tag)
                nc.vector.tensor_scalar_mul(ineg, iv, -1)
                nc.vector.tensor_scalar(ineg, ineg, 1, None, op0=ALU.arith_shift_right)
                nc.vector.tensor_scalar(ineg, ineg, 0x5F3759DF, None, op0=ALU.add)
                y = ineg.bitcast(f32)
                t1 = work.tile(shp, f32, tag="st", bufs=8, name="t1" + tag)
                for _ in range(iters):
                    nc.vector.tensor_mul(t1, y, y)
                    nc.vector.tensor_mul(t1, t1, veps)
                    nc.vector.tensor_scalar(t1, t1, -0.5, 1.5, op0=ALU.mult, op1=ALU.add)
                    nc.vector.tensor_mul(y, y, t1)
                return y

            # ===== phase 2 prologue: groupnorm stats over xallT, xsc98 ====
            xallT_sb = cpool.tile([C, JPC, S], f16, name="xallT_sb")
            if "2" in phases:
                nc.sync.dma_start(out=xallT_sb, in_=xallT_d[:])
                x2t = work.tile([C, JPC, S], f16, tag="x2t", name="x2t")
                nc.scalar.activation(out=x2t, in_=xallT_sb, func=AF.Square)
                s1v = work.tile([98, JPC, G], f32, tag="stv", bufs=8, name="s1v")
                s2v = work.tile([98, JPC, G], f32, tag="stv", bufs=8, name="s2v")
                for g3 in range(3):
                    bs = slice(4 * g3, 4 * (g3 + 1))
                    ps1 = pp.tile([98, 512], f32, tag="pss", bufs=2, name="ps_s1")
                    nc.tensor.matmul(
                        ps1[:, 0:384],
                        lhsT=ones98_sb,
                        rhs=xallT_sb[:, bs, :].rearrange("p b s -> p (b s)"),
                        start=True,
                        stop=True,
                    )
                    nc.vector.tensor_reduce(
                        out=s1v[:, bs, :].unsqueeze(3),
                        in_=ps1[:, 0:384].rearrange("p (b g s) -> p b g s", b=4, g=G),
                        axis=AX.X,
                        op=ALU.add,
                    )
                    ps2 = pp.tile([98, 512], f32, tag="pss", bufs=2, name="ps_s2")
                    nc.tensor.matmul(
                        ps2[:, 0:384],
                        lhsT=ones98_sb,
                        rhs=x2t[:, bs, :].rearrange("p b s -> p (b s)"),
                        start=True,
                        stop=True,
                    )
                    nc.vector.tensor_reduce(
                        out=s2v[:, bs, :].unsqueeze(3),
                        in_=ps2[:, 0:384].rearrange("p (b g s) -> p b g s", b=4, g=G),
                        axis=AX.X,
                        op=ALU.add,
                    )
                mu = work.tile([98, JPC, G], f32, tag="stv", bufs=8, name="mu")
                ex2 = work.tile([98, JPC, G], f32, tag="stv", bufs=8, name="ex2")
                nc.vector.tensor_scalar_mul(mu, s1v, inv_na)
                nc.vector.tensor_scalar_mul(ex2, s2v, inv_na)
                musq = work.tile([98, JPC, G], f32, tag="stv", bufs=8, name="musq")
                nc.vector.tensor_mul(musq, mu, mu)
                veps = work.tile([98, JPC, G], f32, tag="stv", bufs=8, name="veps")
                nc.vector.scalar_tensor_tensor(
                    veps, ex2, EPS, musq, op0=ALU.add, op1=ALU.subtract
                )
                y = newton_rsqrt(veps, "a", iters=2)
                # scale2[p, b, s] = rstd[b, g(s)] * gamma[s]; f16 so the
                # xsc multiply below runs in DVE 2x packed mode
                scale2 = work.tile([98, JPC, S], f16, tag="sc2", name="scale2")
                nc.vector.tensor_tensor(
                    scale2.rearrange("p b (g s) -> p b g s", g=G),
                    y.unsqueeze(3).to_broadcast((98, JPC, G, 12)),
                    gamma_rep.rearrange("p (g s) -> p g s", g=G)
                    .unsqueeze(1)
                    .to_broadcast((98, JPC, G, 12)),
                    ALU.mult,
                )
                # shift2 = beta - mu*scale2 (row 96 feeds the matmul
                # shift-row; rows 0-95 are computed but unused)
                msc = work.tile([98, JPC, S], f16, tag="msc", name="msc")
                nc.vector.tensor_tensor(
                    msc.rearrange("p b (g s) -> p b g s", g=G),
                    mu.unsqueeze(3).to_broadcast((98, JPC, G, 12)),
                    scale2.rearrange("p b (g s) -> p b g s", g=G),
                    ALU.mult,
                )
                shift2 = work.tile([98, JPC, S], f16, tag="sh2", name="shift2")
                nc.vector.tensor_tensor(
                    shift2,
                    beta_rep.unsqueeze(1).to_broadcast((98, JPC, S)),
                    msc,
                    ALU.subtract,
                )
                nc.vector.tensor_mul(xsc98[0:96, :, :], xallT_sb, scale2[0:96, :, :])
                nc.vector.tensor_copy(
                    out=xsc98[96:97, :, :], in_=shift2[96:97, :, :]
                )

            # ===== phase 1: xarr (xn of ALL 96 batches at this core's
            # 12 j-rows, transposed to [l, i, j]) =========================
            xn_s = cpool.tile([B_TOTAL, JPC, C], f16, name="xn_s")
            if "1" in phases:
                xg_sb = work.tile([B_TOTAL, JPC * C], f16, tag="xg", name="xg_sb")
                nc.sync.dma_start(out=xg_sb, in_=xg_d[:])
                sq = work.tile([B_TOTAL, JPC * C], f16, tag="sq", name="sq")
                nc.scalar.activation(out=sq, in_=xg_sb, func=AF.Square)
                s1 = work.tile([C, 1], f32, tag="st", bufs=8, name="s1g")
                s2 = work.tile([C, 1], f32, tag="st", bufs=8, name="s2g")
                nc.vector.tensor_reduce(out=s1, in_=xg_sb, axis=AX.X, op=ALU.add)
                nc.vector.tensor_reduce(out=s2, in_=sq, axis=AX.X, op=ALU.add)
                mu_g = work.tile([C, 1], f32, tag="st", bufs=8, name="mu_g")
                ex2_g = work.tile([C, 1], f32, tag="st", bufs=8, name="ex2_g")
                nc.vector.tensor_scalar_mul(mu_g, s1, inv_na)
                nc.vector.tensor_scalar_mul(ex2_g, s2, inv_na)
                musq_g = work.tile([C, 1], f32, tag="st", bufs=8, name="musq_g")
                nc.vector.tensor_mul(musq_g, mu_g, mu_g)
                veps_g = work.tile([C, 1], f32, tag="st", bufs=8, name="veps_g")
                nc.vector.scalar_tensor_tensor(
                    veps_g, ex2_g, EPS, musq_g, op0=ALU.add, op1=ALU.subtract
                )
                rstd_g = newton_rsqrt(veps_g, "g")
                nc.vector.tensor_scalar(
                    xn_s.rearrange("p j l -> p (j l)"),
                    xg_sb,
                    mu_g,
                    rstd_g,
                    op0=ALU.subtract,
                    op1=ALU.mult,
                )

            def xarr_quad(g):
                if "1" not in phases:
                    return
                for j in range(JW * g, JW * (g + 1)):
                    ps_t = pp.tile(
                        [C, B_TOTAL], f16, tag="pstr", bufs=1, name="ps_tj"
                    )
                    nc.tensor.transpose(ps_t, xn_s[:, j, :], iden_sb)
                    nc.scalar.activation(
                        out=xarr[:, :, j],
                        in_=ps_t,
                        func=AF.Identity,
                        scale=gvec_sb[:, j : j + 1],
                        bias=bb_sb[:, j : j + 1],
                    )

            # ===== qk phase: q/k for all 12 batches, per head ============
            if "2" in phases:
                xsc_bs = xsc98.rearrange("p b s -> p (b s)")
                for h in range(NH):
                    for qi, wsb in ((0, wq_sb), (1, wk_sb)):
                        tA = pp.tile([DK, 1024], f32, tag="ps", bufs=2, name="ps_qk")
                        nc.tensor.matmul(
                            tA[:, 0:384],
                            lhsT=wsb[:, h, :],
                            rhs=xsc_bs[:, 0:384],
                            start=True,
                            stop=True,
                        )
                        nc.tensor.matmul(
                            tA[:, 512:896],
                            lhsT=wsb[:, h, :],
                            rhs=xsc_bs[:, 384:768],
                            start=True,
                            stop=True,
                        )
                        nc.scalar.activation(
                            out=qkT[:, qi, h, 0:8, :].rearrange("p b s -> p (b s)")
                            .rearrange("p (u x) -> p u x", u=2),
                            in_=tA.rearrange("p (u x) -> p u x", u=2)[:, :, 0:384],
                            func=AF.Copy,
                        )
                        tB = pp.tile([DK, 1024], f32, tag="ps", bufs=2, name="ps_qk2")
                        nc.tensor.matmul(
                            tB[:, 0:384],
                            lhsT=wsb[:, h, :],
                            rhs=xsc_bs[:, 768:1152],
                            start=True,
                            stop=True,
                        )
                        nc.scalar.activation(
                            out=qkT[:, qi, h, 8:12, :].rearrange("p b s -> p (b s)"),
                            in_=tB[:, 0:384],
                            func=AF.Copy,
                        )

            # ===== v for all 12 batches (upfront: frees xsc98 early so
            # the next iteration's stats/xsc can overlap this one's
            # assembly tail) ==============================================
            v_all = cpool.tile([S, JPC, NH, DK], f16, name="v_all")
            if "2" in phases:
                for b in range(JPC):
                    psv = pp.tile([S, 1024], f32, tag="ps", bufs=2, name="ps_v")
                    nc.tensor.matmul(
                        psv[:, 0:384],
                        lhsT=xsc98[0:97, b, :],
                        rhs=wv_sb[:, 0:384],
                        start=True,
                        stop=True,
                    )
                    nc.tensor.matmul(
                        psv[:, 512:896],
                        lhsT=xsc98[0:97, b, :],
                        rhs=wv_sb[:, 384:768],
                        start=True,
                        stop=True,
                    )
                    nc.scalar.activation(
                        out=v_all[:, b].rearrange("p (u h) d -> p u (h d)", u=2),
                        in_=psv.rearrange("p (u x) -> p u x", u=2)[:, :, 0:384],
                        func=AF.Copy,
                    )

            # ===== attention stages for the 12 local batches ==============
            st = {}

            def st_s(b):
                d = st[b] = {}
                pst = pp.tile([S, 1024], f32, tag="ps", bufs=2, name="ps_sc")
                for hh in range(2):
                    off = 512 * hh
                    nc.tensor.matmul(
                        pst[:, off : off + 384],
                        lhsT=maskb_sb,
                        rhs=iden4_sb,
                        start=True,
                        stop=False,
                    )
                    for hl in range(4):
                        h = 4 * hh + hl
                        nc.tensor.matmul(
                            pst[:, off + hl * S : off + (hl + 1) * S],
                            lhsT=qkT[:, 1, h, b, :],
                            rhs=qkT[:, 0, h, b, :],
                            start=False,
                            stop=(hl == 3),
                        )
                expT = work.tile([S, NH, S], f16, tag="expT", bufs=4, name="expT")
                nc.scalar.activation(
                    out=expT.rearrange("p (u h) s -> p u (h s)", u=2),
                    in_=pst.rearrange("p (u x) -> p u x", u=2)[:, :, 0:384],
                    func=AF.Exp,
                )
                d["expT"] = expT

            def st_d(b):
                d = st[b]
                expT = d["expT"]
                psd = pp.tile([S, 1024], f32, tag="ps", bufs=2, name="ps_den")
                for hh in range(2):
                    nc.tensor.matmul(
                        psd[:, 512 * hh : 512 * hh + 384],
                        lhsT=ones_sb,
                        rhs=expT[:, 4 * hh : 4 * (hh + 1), :].rearrange(
                            "p h s -> p (h s)"
                        ),
                        start=True,
                        stop=True,
                    )
                recip = work.tile([S, NH, S], f16, tag="recip", bufs=4, name="recip")
                with nc.allow_low_precision(reason="softmax recip f16; 2e-2 gate"):
                    for hh in range(2):
                        nc.vector._custom_dve(
                            RECIPROCAL_APPROX_FAST,
                            out=recip[:, 4 * hh : 4 * (hh + 1), :].rearrange(
                                "p h s -> p (h s)"
                            ),
                            in0=psd[:, 512 * hh : 512 * hh + 384],
                            s0=rc["s0"],
                            s1=rc["s1"],
                            imm2=rc["imm2"],
                        )
                d["recip"] = recip

            def st_o(b):
                d = st[b]
                pso = pp.tile([DK, 1024], f32, tag="ps", bufs=2, name="ps_o")
                for hh in range(2):
                    for hl in range(4):
                        h = 4 * hh + hl
                        nc.tensor.matmul(
                            pso[:, 512 * hh + hl * S : 512 * hh + (hl + 1) * S],
                            lhsT=v_all[:, b, h, :],
                            rhs=d["expT"][:, h, :],
                            start=True,
                            stop=True,
                        )
                oraw = work.tile([DK, NH, S], f16, tag="oraw", bufs=4, name="oraw")
                nc.scalar.activation(
                    out=oraw.rearrange("p (u h) s -> p u (h s)", u=2),
                    in_=pso.rearrange("p (u x) -> p u x", u=2)[:, :, 0:384],
                    func=AF.Copy,
                )
                ocatT = work.tile([DK, NH, S], f16, tag="ocatT", bufs=4, name="ocatT")
                with nc.allow_low_precision(reason="attn weights f16; 2e-2 gate"):
                    nc.gpsimd.tensor_tensor(ocatT, oraw, d["recip"], ALU.mult)
                d["ocatT"] = ocatT

            def st_w(b):
                d = st.pop(b)
                psw = pp.tile([C, S], f32, tag="psw", bufs=1, name="ps_w")
                for h in range(NH):
                    nc.tensor.matmul(
                        psw,
                        lhsT=wo_sb[:, h, :],
                        rhs=d["ocatT"][:, h, :],
                        start=(h == 0),
                        stop=(h == NH - 1),
                    )
                nc.scalar.activation(out=aarr[:, :, b], in_=psw, func=AF.Copy)

            # ===== assembly quad g: res[l, i24, k, j4] = A + X ===========
            dma_eng = nc.scalar if dma_ring == "scalar" else nc.sync

            def asm_quad(g):
                if "3" not in phases and "4" not in phases:
                    return
                js = slice(JW * g, JW * (g + 1))
                for ic in range(JW):
                    res = work.tile(
                        [C, IW, S, JW], f16, tag="res", bufs=3, name="res"
                    )
                    nc.vector.tensor_tensor(
                        res,
                        aarr[:, :, js].unsqueeze(1).to_broadcast((C, IW, S, JW)),
                        xarr[:, ic * IW : (ic + 1) * IW, js].unsqueeze(2).to_broadcast(
                            (C, IW, S, JW)
                        ),
                        ALU.add,
                    )
                    if "4" not in phases:
                        dma_eng.dma_start(out=out_d[g, ic], in_=res)

            # ===== schedule: quads of 4 batches, stage-interleaved; the
            # assembly of quad g-1 is emitted inside quad g's stages so
            # DVE chews on it while PE/ACT run quad g ====================
            if "2" in phases:
                for g in range(NJG):
                    xarr_quad(g)
                    bs = [4 * g + i for i in range(4)]
                    for fn in (st_s, st_d):
                        for b in bs:
                            fn(b)
                    if g > 0:
                        asm_quad(g - 1)
                    for fn in (st_o, st_w):
                        for b in bs:
                            fn(b)
                asm_quad(NJG - 1)
            elif "1" in phases or "3" in phases or "4" in phases:
                # assembly/DMA timing variants without attention
                nc.vector.memset(aarr[:], 0.0)
                if "1" not in phases and ("3" in phases or "4" in phases):
                    nc.vector.memset(xarr[:], 0.0)
                for g in range(NJG):
                    xarr_quad(g)
                    asm_quad(g)

            # DMA probe "z": out-DMA only, from one dummy buffer
            if "z" in phases:
                dm = cpool.tile([C, IW, S, JW], f16, name="dummy_res")
                nc.vector.memset(dm[:], 0.25)
                for g in range(NJG):
                    for ic in range(JW):
                        dma_eng.dma_start(out=out_d[g, ic], in_=dm)

            loop_cm.__exit__(None, None, None)

    nc.compile()
    return nc


def _get_program():
    global _PROG
    if _PROG is None:
        _PROG = _build_program()
    return _PROG


def _host_inputs(x, Wq, bq, Wk, bk, Wv, bv, Wo, bo, gamma, beta):
    f32 = np.float32
    f16 = np.float16
    x = np.asarray(x, f32)
    Wq = np.asarray(Wq, f32)
    bq = np.asarray(bq, f32)
    Wk = np.asarray(Wk, f32)
    bk = np.asarray(bk, f32)
    Wv = np.asarray(Wv, f32)
    bv = np.asarray(bv, f32)
    Wo = np.asarray(Wo, f32)
    bo = np.asarray(bo, f32)
    gamma = np.asarray(gamma, f32)
    beta = np.asarray(beta, f32)

    sc = f32(1.0 / np.sqrt(DK))
    bo_eff = (bv.astype(np.float64) @ Wo.astype(np.float64) + bo).astype(f32)

    cp16 = np.zeros((98, 3936), f16)
    cp16[0:96, 0:768] = (Wq * sc).astype(f16)
    cp16[96, 0:768] = (Wq.sum(axis=0) * sc).astype(f16)
    cp16[97, 0:768] = (bq * sc).astype(f16)
    cp16[0:96, 768:1536] = Wk.astype(f16)
    cp16[96, 768:1536] = Wk.sum(axis=0).astype(f16)
    cp16[97, 768:1536] = bk.astype(f16)
    cp16[0:96, 1536:2304] = Wv.astype(f16)
    cp16[96, 1536:2304] = Wv.sum(axis=0).astype(f16)
    cp16[0:96, 2304:3072] = (
        Wo.reshape(NH, DK, C).transpose(1, 0, 2).reshape(DK, 768).astype(f16)
    )
    # maskbT[q, t] = -30 where t > q (causal), added to scoresT in-psum
    cp16[0:S, 3072:3168] = np.triu(np.full((S, S), -30.0, f16), 1)
    cp16[0:S, 3168:3264] = np.ones((S, S), f16)
    cp16[0:C, 3264:3648] = np.broadcast_to(
        np.eye(C, dtype=f16)[:, None, :], (C, 4, S)
    ).reshape(C, 384)
    cp16[0:C, 3648:3744] = np.eye(C, dtype=f16)
    cp16[0:C, 3744:3842] = np.ones((C, 98), f16)

    com = {"cpack16": cp16}
    x_r = np.ascontiguousarray(x.reshape(B_TOTAL, C, C))
    in_maps = []
    for c in range(NCORES):
        J = slice(c * JPC, (c + 1) * JPC)
        m = dict(com)
        # xallT[c, b, s] = x[local b, s(chan), c(w)] transposed
        m["xallT"] = np.ascontiguousarray(
            x_r[J].transpose(2, 0, 1).astype(f16)
        )
        m["xg"] = (
            np.ascontiguousarray(x_r[:, J, :])
            .reshape(B_TOTAL, JPC * C)
            .astype(f16)
        )
        cp32 = np.zeros((98, 216), f32)
        cp32[0:C, 0:12] = np.broadcast_to(gamma[J][None, :], (C, JPC))
        cp32[0:C, 12:24] = beta[J][None, :] + bo_eff[:, None]
        cp32[:, 24:120] = np.broadcast_to(gamma[None, :], (98, C))
        cp32[:, 120:216] = np.broadcast_to(beta[None, :], (98, C))
        m["cpack32"] = cp32
        in_maps.append(m)
    return in_maps


def _assemble(parts):
    """parts[c]: (NJG, JW, C, IW, S, JW) f16 [jq, ic, l, i24, k, j4]
    -> (B, C, S, C) f32."""
    cols = []
    for a in parts:
        a = np.asarray(a).astype(np.float32).reshape(NJG, JW, C, IW, S, JW)
        # (jq, ic, l, i24, k, j4) -> (ic, i24, jq, j4, k, l)
        a = a.transpose(1, 3, 0, 5, 4, 2).reshape(B_TOTAL, JPC, S, C)
        cols.append(a)
    return np.concatenate(cols, axis=1)


def _run(inputs, trace=False):
    from concourse.bass_utils import run_bass_kernel_spmd

    nc = _get_program()
    in_maps = _host_inputs(**inputs)
    res = run_bass_kernel_spmd(
        nc, in_maps, core_ids=list(range(NCORES)), trace=trace
    )
    out = _assemble([r["out"] for r in res.results])
    return out, res


def kernel(**inputs) -> np.ndarray:
    out, _ = _run(inputs, trace=False)
    return out


# revision 17
# speedup vs baseline: 1.0550x; 1.0550x over previous
"""Trainium2 Bass kernel for nn_Attention_43542378447097 (v3).

GroupNorm -> multi-head causal self-attention -> out-proj, then the
reference's broadcast add:

    out(B,S,C) + residual(B,C,1,C)  ->  (B,C,S,C)

    result[i, j, k, l] = A[j, k, l] + xn[i, j, l]

where A[j] = attention output (no bo) of batch j and xn[i] = groupnorm
output of batch i (+ beta + bo_eff folded in).  Output (96,96,96,96),
fp16 on device = 21.2MB/core across 8 j-sharded cores -> the out-DMA
(~71us on the scalar HWDGE ring) is the intended critical path.

v3 vs v2 (measured on HW via probe phases):
  * Assembly TT reshaped to j-quad innermost [l, i24, k96, j4]: DVE
    hits ~2.2 elem/cyc/partition (54.7us total) vs 1.2 for j-pairs
    (96us).  aarr is [l, k, j] and xarr [l, i, j] so both operands
    stream innermost step-1 quads; broadcasts sit on mid dims only.
  * Out-DMAs issued on nc.scalar (qActDynamicHW ring): 71.2us vs
    84.6us on nc.sync for the same 12 x 1.77MB transfers.
  * Attention restructured around xallT [c, b, s] (host-transposed):
    groupnorm scale folds into the contraction as xsc98 = xallT *
    scale2 with two extra matmul rows (shift-row, ones-row) carrying
    the shift*Wsum and bias terms, so q/k/v need no per-batch
    transposes or ACT scale passes.  q/k are batched over all 12
    local batches per head (N=384 matmuls).
  * Softmax: denominators via ones-matmul (replicated across
    partitions), reciprocal via the custom DVE op (~1 cyc/elem, f16
    out), and the normalize multiply runs on the otherwise-idle
    GPSIMD engine from SBUF (ACT evicts raw oT).
  * ACT only ever loads the exp_and_others table (Copy/Identity/Exp/
    Square); rsqrt stays the all-DVE Newton chain.
"""

import sys

sys.path.insert(0, "/opt/trn_rl_repo")

import numpy as np

B_TOTAL = 96
C = 96
S = 96
NH = 8
DK = 96
G = 8
NCORES = 8
JPC = B_TOTAL // NCORES  # 12 local j's (attention batches) per core
JW = 4  # assembly quad width (j's per asm group)
NJG = JPC // JW  # 3 quad groups
IW = 96 // JW  # 24 i's per asm op
EPS = 1e-5

_PROG = None


def _build_program(loop_n=1, phases="123", aspl=None, dma_ring="scalar"):
    import contextlib

    import concourse.bass as bass
    import concourse.tile as tile
    from concourse import bacc, mybir
    from concourse.dve_ops import RECIP_APPROX_FAST_CONSTS, RECIPROCAL_APPROX_FAST

    f32 = mybir.dt.float32
    f16 = mybir.dt.float16
    i32 = mybir.dt.int32
    AF = mybir.ActivationFunctionType
    ALU = mybir.AluOpType
    AX = mybir.AxisListType

    nc = bacc.Bacc(
        "TRN2",
        target_bir_lowering=False,
        debug=False,
        enable_asserts=False,
        num_devices=NCORES,
    )

    xallT_d = nc.declare_dram_parameter("xallT", [C, JPC, S], f16, isOutput=False)
    xg_d = nc.declare_dram_parameter("xg", [B_TOTAL, JPC * C], f16, isOutput=False)
    cp16_d = nc.declare_dram_parameter("cpack16", [98, 3936], f16, isOutput=False)
    cp32_d = nc.declare_dram_parameter("cpack32", [98, 216], f32, isOutput=False)
    out_d = nc.declare_dram_parameter(
        "out", [NJG, JW, C, IW, S, JW], f16, isOutput=True
    )

    with tile.TileContext(nc) as tc:
        with (
            tc.tile_pool(name="const", bufs=1) as cpool,
            tc.tile_pool(name="work", bufs=2) as work,
            tc.tile_pool(name="psum", bufs=8, space="PSUM") as pp,
        ):
            # ---- constants (packed tiles; views below) ----
            cp16 = cpool.tile([98, 3936], f16, name="cp16")
            cp32 = cpool.tile([98, 216], f32, name="cp32")
            aarr = cpool.tile([C, S, JPC], f16, name="aarr")  # [l, k, j]
            xarr = cpool.tile([C, B_TOTAL, JPC], f16, name="xarr")  # [l, i, j]
            xsc98 = cpool.tile([98, JPC, S], f16, name="xsc98")
            qkT = cpool.tile([DK, 2, NH, JPC, S], f16, name="qkT")

            wq_sb = cp16[:, 0:768].rearrange("p (h d) -> p h d", h=NH)
            wk_sb = cp16[:, 768:1536].rearrange("p (h d) -> p h d", h=NH)
            wv_sb = cp16[0:97, 1536:2304]
            wo_sb = cp16[0:DK, 2304:3072].rearrange("p (h l) -> p h l", h=NH)
            maskb_sb = cp16[0:S, 3072:3168]
            ones_sb = cp16[0:S, 3168:3264]
            iden4_sb = cp16[0:C, 3264:3648]
            iden_sb = cp16[0:C, 3648:3744]
            ones98_sb = cp16[0:C, 3744:3842]
            gvec_sb = cp32[0:C, 0:12]
            bb_sb = cp32[0:C, 12:24]
            gamma_rep = cp32[:, 24:120]
            beta_rep = cp32[:, 120:216]

            nc.sync.dma_start(out=cp16, in_=cp16_d[:])
            nc.sync.dma_start(out=cp32, in_=cp32_d[:])
            # rows 96-97 = 1.0 once; the loop overwrites row 96 with the
            # shift row each iteration (engines need 32-aligned start
            # partitions, so the two rows are set together)
            nc.vector.memset(xsc98[96:98, :, :], 1.0)

            inv_na = 1.0 / (C * C // G)  # 1/1152 per (batch, group)
            rc = RECIP_APPROX_FAST_CONSTS

            loop_cm = (
                tc.For_i(0, loop_n, 1, staggered_reset=True)
                if loop_n > 1
                else contextlib.nullcontext()
            )
            loop_cm.__enter__()

            def newton_rsqrt(veps, tag, iters=2):
                """rstd = rsqrt(veps), all-DVE (quake seed + Newton steps)
                so ACT only ever needs the Exp table set."""
                shp = list(veps.shape)
                iv = veps.bitcast(i32)
                ineg = work.tile(shp, i32, tag="sti", bufs=8, name="ineg" + tag)
                nc.vector.tensor_scalar_mul(ineg, iv, -1)
                nc.vector.tensor_scalar(ineg, ineg, 1, None, op0=ALU.arith_shift_right)
                nc.vector.tensor_scalar(ineg, ineg, 0x5F3759DF, None, op0=ALU.add)
                y = ineg.bitcast(f32)
                t1 = work.tile(shp, f32, tag="st", bufs=8, name="t1" + tag)
                for _ in range(iters):
                    nc.vector.tensor_mul(t1, y, y)
                    nc.vector.tensor_mul(t1, t1, veps)
                    nc.vector.tensor_scalar(t1, t1, -0.5, 1.5, op0=ALU.mult, op1=ALU.add)
                    nc.vector.tensor_mul(y, y, t1)
                return y

            # ===== phase 2 prologue: groupnorm stats over xallT, xsc98 ====
            xallT_sb = cpool.tile([C, JPC, S], f16, name="xallT_sb")
            if "2" in phases:
                nc.sync.dma_start(out=xallT_sb, in_=xallT_d[:])
                x2t = work.tile([C, JPC, S], f16, tag="x2t", name="x2t")
                nc.scalar.activation(out=x2t, in_=xallT_sb, func=AF.Square)
                s1v = work.tile([98, JPC, G], f32, tag="stv", bufs=8, name="s1v")
                s2v = work.tile([98, JPC, G], f32, tag="stv", bufs=8, name="s2v")
                for g3 in range(3):
                    bs = slice(4 * g3, 4 * (g3 + 1))
                    ps1 = pp.tile([98, 512], f32, tag="pss", bufs=2, name="ps_s1")
                    nc.tensor.matmul(
                        ps1[:, 0:384],
                        lhsT=ones98_sb,
                        rhs=xallT_sb[:, bs, :].rearrange("p b s -> p (b s)"),
                        start=True,
                        stop=True,
                    )
                    nc.vector.tensor_reduce(
                        out=s1v[:, bs, :].unsqueeze(3),
                        in_=ps1[:, 0:384].rearrange("p (b g s) -> p b g s", b=4, g=G),
                        axis=AX.X,
                        op=ALU.add,
                    )
                    ps2 = pp.tile([98, 512], f32, tag="pss", bufs=2, name="ps_s2")
                    nc.tensor.matmul(
                        ps2[:, 0:384],
                        lhsT=ones98_sb,
                        rhs=x2t[:, bs, :].rearrange("p b s -> p (b s)"),
                        start=True,
                        stop=True,
                    )
                    nc.vector.tensor_reduce(
                        out=s2v[:, bs, :].unsqueeze(3),
                        in_=ps2[:, 0:384].rearrange("p (b g s) -> p b g s", b=4, g=G),
                        axis=AX.X,
                        op=ALU.add,
                    )
                mu = work.tile([98, JPC, G], f32, tag="stv", bufs=8, name="mu")
                ex2 = work.tile([98, JPC, G], f32, tag="stv", bufs=8, name="ex2")
                nc.vector.tensor_scalar_mul(mu, s1v, inv_na)
                nc.vector.tensor_scalar_mul(ex2, s2v, inv_na)
                musq = work.tile([98, JPC, G], f32, tag="stv", bufs=8, name="musq")
                nc.vector.tensor_mul(musq, mu, mu)
                veps = work.tile([98, JPC, G], f32, tag="stv", bufs=8, name="veps")
                nc.vector.scalar_tensor_tensor(
                    veps, ex2, EPS, musq, op0=ALU.add, op1=ALU.subtract
                )
                y = newton_rsqrt(veps, "a", iters=2)
                # scale2[p, b, s] = rstd[b, g(s)] * gamma[s]; f16 so the
                # xsc multiply below runs in DVE 2x packed mode
                scale2 = work.tile([98, JPC, S], f16, tag="sc2", name="scale2")
                nc.vector.tensor_tensor(
                    scale2.rearrange("p b (g s) -> p b g s", g=G),
                    y.unsqueeze(3).to_broadcast((98, JPC, G, 12)),
                    gamma_rep.rearrange("p (g s) -> p g s", g=G)
                    .unsqueeze(1)
                    .to_broadcast((98, JPC, G, 12)),
                    ALU.mult,
                )
                # shift2 = beta - mu*scale2 (row 96 feeds the matmul
                # shift-row; rows 0-95 are computed but unused)
                msc = work.tile([98, JPC, S], f16, tag="msc", name="msc")
                nc.vector.tensor_tensor(
                    msc.rearrange("p b (g s) -> p b g s", g=G),
                    mu.unsqueeze(3).to_broadcast((98, JPC, G, 12)),
                    scale2.rearrange("p b (g s) -> p b g s", g=G),
                    ALU.mult,
                )
                shift2 = work.tile([98, JPC, S], f16, tag="sh2", name="shift2")
                nc.vector.tensor_tensor(
                    shift2,
                    beta_rep.unsqueeze(1).to_broadcast((98, JPC, S)),
                    msc,
                    ALU.subtract,
                )
                nc.vector.tensor_mul(xsc98[0:96, :, :], xallT_sb, scale2[0:96, :, :])
                nc.vector.tensor_copy(
                    out=xsc98[96:97, :, :], in_=shift2[96:97, :, :]
                )

            # ===== phase 1: xarr (xn of ALL 96 batches at this core's
            # 12 j-rows, transposed to [l, i, j]) =========================
            xn_s = cpool.tile([B_TOTAL, JPC, C], f16, name="xn_s")
            if "1" in phases:
                xg_sb = work.tile([B_TOTAL, JPC * C], f16, tag="xg", name="xg_sb")
                nc.sync.dma_start(out=xg_sb, in_=xg_d[:])
                sq = work.tile([B_TOTAL, JPC * C], f16, tag="sq", name="sq")
                nc.scalar.activation(out=sq, in_=xg_sb, func=AF.Square)
                s1 = work.tile([C, 1], f32, tag="st", bufs=8, name="s1g")
                s2 = work.tile([C, 1], f32, tag="st", bufs=8, name="s2g")
                nc.vector.tensor_reduce(out=s1, in_=xg_sb, axis=AX.X, op=ALU.add)
                nc.vector.tensor_reduce(out=s2, in_=sq, axis=AX.X, op=ALU.add)
                mu_g = work.tile([C, 1], f32, tag="st", bufs=8, name="mu_g")
                ex2_g = work.tile([C, 1], f32, tag="st", bufs=8, name="ex2_g")
                nc.vector.tensor_scalar_mul(mu_g, s1, inv_na)
                nc.vector.tensor_scalar_mul(ex2_g, s2, inv_na)
                musq_g = work.tile([C, 1], f32, tag="st", bufs=8, name="musq_g")
                nc.vector.tensor_mul(musq_g, mu_g, mu_g)
                veps_g = work.tile([C, 1], f32, tag="st", bufs=8, name="veps_g")
                nc.vector.scalar_tensor_tensor(
                    veps_g, ex2_g, EPS, musq_g, op0=ALU.add, op1=ALU.subtract
                )
                rstd_g = newton_rsqrt(veps_g, "g")
                nc.vector.tensor_scalar(
                    xn_s.rearrange("p j l -> p (j l)"),
                    xg_sb,
                    mu_g,
                    rstd_g,
                    op0=ALU.subtract,
                    op1=ALU.mult,
                )

            def xarr_quad(g):
                if "1" not in phases:
                    return
                for j in range(JW * g, JW * (g + 1)):
                    ps_t = pp.tile(
                        [C, B_TOTAL], f16, tag="pstr", bufs=1, name="ps_tj"
                    )
                    nc.tensor.transpose(ps_t, xn_s[:, j, :], iden_sb)
                    nc.scalar.activation(
                        out=xarr[:, :, j],
                        in_=ps_t,
                        func=AF.Identity,
                        scale=gvec_sb[:, j : j + 1],
                        bias=bb_sb[:, j : j + 1],
                    )

            # ===== per-quad q/k (batched over the quad's 4 batches) and
            # v: emitted at each quad's top so quad 0's scores start as
            # early as possible and later quads' projections overlap
            # earlier quads' stages on ACT slack ==========================
            v_all = cpool.tile([S, JPC, NH, DK], f16, name="v_all")
            xsc_bs = xsc98.rearrange("p b s -> p (b s)")

            def qkv_quad(g):
                lo = 384 * g
                for h in range(NH):
                    tA = pp.tile([DK, 1024], f32, tag="ps", bufs=2, name="ps_qk")
                    for qi, wsb in ((0, wq_sb), (1, wk_sb)):
                        nc.tensor.matmul(
                            tA[:, 512 * qi : 512 * qi + 384],
                            lhsT=wsb[:, h, :],
                            rhs=xsc_bs[:, lo : lo + 384],
                            start=True,
                            stop=True,
                        )
                    nc.scalar.activation(
                        out=qkT[:, :, h, 4 * g : 4 * (g + 1), :],
                        in_=tA.rearrange("p (u x) -> p u x", u=2)[
                            :, :, 0:384
                        ].rearrange("p u (b s) -> p u b s", b=4),
                        func=AF.Copy,
                    )
                for b in range(4 * g, 4 * (g + 1)):
                    psv = pp.tile([S, 1024], f32, tag="ps", bufs=2, name="ps_v")
                    nc.tensor.matmul(
                        psv[:, 0:384],
                        lhsT=xsc98[0:97, b, :],
                        rhs=wv_sb[:, 0:384],
                        start=True,
                        stop=True,
                    )
                    nc.tensor.matmul(
                        psv[:, 512:896],
                        lhsT=xsc98[0:97, b, :],
                        rhs=wv_sb[:, 384:768],
                        start=True,
                        stop=True,
                    )
                    nc.scalar.activation(
                        out=v_all[:, b].rearrange("p (u h) d -> p u (h d)", u=2),
                        in_=psv.rearrange("p (u x) -> p u x", u=2)[:, :, 0:384],
                        func=AF.Copy,
                    )

            # ===== attention stages for the 12 local batches ==============
            st = {}

            def st_s(b):
                d = st[b] = {}
                pst = pp.tile([S, 1024], f32, tag="ps", bufs=2, name="ps_sc")
                for hh in range(2):
                    off = 512 * hh
                    nc.tensor.matmul(
                        pst[:, off : off + 384],
                        lhsT=maskb_sb,
                        rhs=iden4_sb,
                        start=True,
                        stop=False,
                    )
                    for hl in range(4):
                        h = 4 * hh + hl
                        nc.tensor.matmul(
                            pst[:, off + hl * S : off + (hl + 1) * S],
                            lhsT=qkT[:, 1, h, b, :],
                            rhs=qkT[:, 0, h, b, :],
                            start=False,
                            stop=(hl == 3),
                        )
                expT = work.tile([S, NH, S], f16, tag="expT", bufs=4, name="expT")
                nc.scalar.activation(
                    out=expT.rearrange("p (u h) s -> p u (h s)", u=2),
                    in_=pst.rearrange("p (u x) -> p u x", u=2)[:, :, 0:384],
                    func=AF.Exp,
                )
                d["expT"] = expT

            def st_d(b):
                d = st[b]
                expT = d["expT"]
                psd = pp.tile([S, 1024], f32, tag="ps", bufs=2, name="ps_den")
                for hh in range(2):
                    nc.tensor.matmul(
                        psd[:, 512 * hh : 512 * hh + 384],
                        lhsT=ones_sb,
                        rhs=expT[:, 4 * hh : 4 * (hh + 1), :].rearrange(
                            "p h s -> p (h s)"
                        ),
                        start=True,
                        stop=True,
                    )
                recip = work.tile([S, NH, S], f16, tag="recip", bufs=4, name="recip")
                with nc.allow_low_precision(reason="softmax recip f16; 2e-2 gate"):
                    for hh in range(2):
                        nc.vector._custom_dve(
                            RECIPROCAL_APPROX_FAST,
                            out=recip[:, 4 * hh : 4 * (hh + 1), :].rearrange(
                                "p h s -> p (h s)"
                            ),
                            in0=psd[:, 512 * hh : 512 * hh + 384],
                            s0=rc["s0"],
                            s1=rc["s1"],
                            imm2=rc["imm2"],
                        )
                d["recip"] = recip

            def st_o(b):
                d = st[b]
                pso = pp.tile([DK, 1024], f32, tag="ps", bufs=2, name="ps_o")
                for hh in range(2):
                    for hl in range(4):
                        h = 4 * hh + hl
                        nc.tensor.matmul(
                            pso[:, 512 * hh + hl * S : 512 * hh + (hl + 1) * S],
                            lhsT=v_all[:, b, h, :],
                            rhs=d["expT"][:, h, :],
                            start=True,
                            stop=True,
                        )
                oraw = work.tile([DK, NH, S], f16, tag="oraw", bufs=4, name="oraw")
                nc.scalar.activation(
                    out=oraw.rearrange("p (u h) s -> p u (h s)", u=2),
                    in_=pso.rearrange("p (u x) -> p u x", u=2)[:, :, 0:384],
                    func=AF.Copy,
                )
                ocatT = work.tile([DK, NH, S], f16, tag="ocatT", bufs=4, name="ocatT")
                with nc.allow_low_precision(reason="attn weights f16; 2e-2 gate"):
                    nc.gpsimd.tensor_tensor(ocatT, oraw, d["recip"], ALU.mult)
                d["ocatT"] = ocatT

            def st_w(b):
                d = st.pop(b)
                psw = pp.tile([C, S], f32, tag="psw", bufs=1, name="ps_w")
                for h in range(NH):
                    nc.tensor.matmul(
                        psw,
                        lhsT=wo_sb[:, h, :],
                        rhs=d["ocatT"][:, h, :],
                        start=(h == 0),
                        stop=(h == NH - 1),
                    )
                nc.scalar.activation(out=aarr[:, :, b], in_=psw, func=AF.Copy)

            # ===== assembly quad g: res[l, i24, k, j4] = A + X ===========
            dma_eng = nc.scalar if dma_ring == "scalar" else nc.sync

            def asm_quad(g):
                if "3" not in phases and "4" not in phases:
                    return
                js = slice(JW * g, JW * (g + 1))
                for ic in range(JW):
                    res = work.tile(
                        [C, IW, S, JW], f16, tag="res", bufs=3, name="res"
                    )
                    nc.vector.tensor_tensor(
                        res,
                        aarr[:, :, js].unsqueeze(1).to_broadcast((C, IW, S, JW)),
                        xarr[:, ic * IW : (ic + 1) * IW, js].unsqueeze(2).to_broadcast(
                            (C, IW, S, JW)
                        ),
                        ALU.add,
                    )
                    if "4" not in phases:
                        dma_eng.dma_start(out=out_d[g, ic], in_=res)

            # ===== schedule: quads of 4 batches, stage-interleaved; the
            # assembly of quad g-1 is emitted inside quad g's stages so
            # DVE chews on it while PE/ACT run quad g ====================
            if "2" in phases:
                for g in range(NJG):
                    qkv_quad(g)
                    xarr_quad(g)
                    bs = [4 * g + i for i in range(4)]
                    for fn in (st_s, st_d, st_o, st_w):
                        for b in bs:
                            fn(b)
                    asm_quad(g)
            elif "1" in phases or "3" in phases or "4" in phases:
                # assembly/DMA timing variants without attention
                nc.vector.memset(aarr[:], 0.0)
                if "1" not in phases and ("3" in phases or "4" in phases):
                    nc.vector.memset(xarr[:], 0.0)
                for g in range(NJG):
                    xarr_quad(g)
                    asm_quad(g)

            # DMA probe "z": out-DMA only, from one dummy buffer
            if "z" in phases:
                dm = cpool.tile([C, IW, S, JW], f16, name="dummy_res")
                nc.vector.memset(dm[:], 0.25)
                for g in range(NJG):
                    for ic in range(JW):
                        dma_eng.dma_start(out=out_d[g, ic], in_=dm)

            loop_cm.__exit__(None, None, None)

    nc.compile()
    return nc


def _get_program():
    global _PROG
    if _PROG is None:
        _PROG = _build_program()
    return _PROG


def _host_inputs(x, Wq, bq, Wk, bk, Wv, bv, Wo, bo, gamma, beta):
    f32 = np.float32
    f16 = np.float16
    x = np.asarray(x, f32)
    Wq = np.asarray(Wq, f32)
    bq = np.asarray(bq, f32)
    Wk = np.asarray(Wk, f32)
    bk = np.asarray(bk, f32)
    Wv = np.asarray(Wv, f32)
    bv = np.asarray(bv, f32)
    Wo = np.asarray(Wo, f32)
    bo = np.asarray(bo, f32)
    gamma = np.asarray(gamma, f32)
    beta = np.asarray(beta, f32)

    sc = f32(1.0 / np.sqrt(DK))
    bo_eff = (bv.astype(np.float64) @ Wo.astype(np.float64) + bo).astype(f32)

    cp16 = np.zeros((98, 3936), f16)
    cp16[0:96, 0:768] = (Wq * sc).astype(f16)
    cp16[96, 0:768] = (Wq.sum(axis=0) * sc).astype(f16)
    cp16[97, 0:768] = (bq * sc).astype(f16)
    cp16[0:96, 768:1536] = Wk.astype(f16)
    cp16[96, 768:1536] = Wk.sum(axis=0).astype(f16)
    cp16[97, 768:1536] = bk.astype(f16)
    cp16[0:96, 1536:2304] = Wv.astype(f16)
    cp16[96, 1536:2304] = Wv.sum(axis=0).astype(f16)
    cp16[0:96, 2304:3072] = (
        Wo.reshape(NH, DK, C).transpose(1, 0, 2).reshape(DK, 768).astype(f16)
    )
    # maskbT[q, t] = -30 where t > q (causal), added to scoresT in-psum
    cp16[0:S, 3072:3168] = np.triu(np.full((S, S), -30.0, f16), 1)
    cp16[0:S, 3168:3264] = np.ones((S, S), f16)
    cp16[0:C, 3264:3648] = np.broadcast_to(
        np.eye(C, dtype=f16)[:, None, :], (C, 4, S)
    ).reshape(C, 384)
    cp16[0:C, 3648:3744] = np.eye(C, dtype=f16)
    cp16[0:C, 3744:3842] = np.ones((C, 98), f16)

    com = {"cpack16": cp16}
    x_r = np.ascontiguousarray(x.reshape(B_TOTAL, C, C))
    in_maps = []
    for c in range(NCORES):
        J = slice(c * JPC, (c + 1) * JPC)
        m = dict(com)
        # xallT[c, b, s] = x[local b, s(chan), c(w)] transposed
        m["xallT"] = np.ascontiguousarray(
            x_r[J].transpose(2, 0, 1).astype(f16)
        )
        m["xg"] = (
            np.ascontiguousarray(x_r[:, J, :])
            .reshape(B_TOTAL, JPC * C)
            .astype(f16)
        )
        cp32 = np.zeros((98, 216), f32)
        cp32[0:C, 0:12] = np.broadcast_to(gamma[J][None, :], (C, JPC))
        cp32[0:C, 12:24] = beta[J][None, :] + bo_eff[:, None]
        cp32[:, 24:120] = np.broadcast_to(gamma[None, :], (98, C))
        cp32[:, 120:216] = np.broadcast_to(beta[None, :], (98, C))
        m["cpack32"] = cp32
        in_maps.append(m)
    return in_maps


def _assemble(parts):
    """parts[c]: (NJG, JW, C, IW, S, JW) f16 [jq, ic, l, i24, k, j4]
    -> (B, C, S, C) f32."""
    cols = []
    for a in parts:
        a = np.asarray(a).astype(np.float32).reshape(NJG, JW, C, IW, S, JW)
        # (jq, ic, l, i24, k, j4) -> (ic, i24, jq, j4, k, l)
        a = a.transpose(1, 3, 0, 5, 4, 2).reshape(B_TOTAL, JPC, S, C)
        cols.append(a)
    return np.concatenate(cols, axis=1)


def _run(inputs, trace=False):
    from concourse.bass_utils import run_bass_kernel_spmd

    nc = _get_program()
    in_maps = _host_inputs(**inputs)
    res = run_bass_kernel_spmd(
        nc, in_maps, core_ids=list(range(NCORES)), trace=trace
    )
    out = _assemble([r["out"] for r in res.results])
    return out, res


def kernel(**inputs) -> np.ndarray:
    out, _ = _run(inputs, trace=False)
    return out


# revision 18
# speedup vs baseline: 1.0726x; 1.0167x over previous
"""Trainium2 Bass kernel for nn_Attention_43542378447097 (v3).

GroupNorm -> multi-head causal self-attention -> out-proj, then the
reference's broadcast add:

    out(B,S,C) + residual(B,C,1,C)  ->  (B,C,S,C)

    result[i, j, k, l] = A[j, k, l] + xn[i, j, l]

where A[j] = attention output (no bo) of batch j and xn[i] = groupnorm
output of batch i (+ beta + bo_eff folded in).  Output (96,96,96,96),
fp16 on device = 21.2MB/core across 8 j-sharded cores -> the out-DMA
(~71us on the scalar HWDGE ring) is the intended critical path.

v3 vs v2 (measured on HW via probe phases):
  * Assembly TT reshaped to j-quad innermost [l, i24, k96, j4]: DVE
    hits ~2.2 elem/cyc/partition (54.7us total) vs 1.2 for j-pairs
    (96us).  aarr is [l, k, j] and xarr [l, i, j] so both operands
    stream innermost step-1 quads; broadcasts sit on mid dims only.
  * Out-DMAs issued on nc.scalar (qActDynamicHW ring): 71.2us vs
    84.6us on nc.sync for the same 12 x 1.77MB transfers.
  * Attention restructured around xallT [c, b, s] (host-transposed):
    groupnorm scale folds into the contraction as xsc98 = xallT *
    scale2 with two extra matmul rows (shift-row, ones-row) carrying
    the shift*Wsum and bias terms, so q/k/v need no per-batch
    transposes or ACT scale passes.  q/k are batched over all 12
    local batches per head (N=384 matmuls).
  * Softmax: denominators via ones-matmul (replicated across
    partitions), reciprocal via the custom DVE op (~1 cyc/elem, f16
    out), and the normalize multiply runs on the otherwise-idle
    GPSIMD engine from SBUF (ACT evicts raw oT).
  * ACT only ever loads the exp_and_others table (Copy/Identity/Exp/
    Square); rsqrt stays the all-DVE Newton chain.
"""

import sys

sys.path.insert(0, "/opt/trn_rl_repo")

import numpy as np

B_TOTAL = 96
C = 96
S = 96
NH = 8
DK = 96
G = 8
NCORES = 8
JPC = B_TOTAL // NCORES  # 12 local j's (attention batches) per core
JW = 4  # assembly quad width (j's per asm group)
NJG = JPC // JW  # 3 quad groups
IW = 96 // JW  # 24 i's per asm op
EPS = 1e-5

_PROG = None


def _build_program(loop_n=1, phases="123", aspl=None, dma_ring="scalar"):
    import contextlib

    import concourse.bass as bass
    import concourse.tile as tile
    from concourse import bacc, mybir
    from concourse.dve_ops import RECIP_APPROX_FAST_CONSTS, RECIPROCAL_APPROX_FAST

    f32 = mybir.dt.float32
    f16 = mybir.dt.float16
    i32 = mybir.dt.int32
    AF = mybir.ActivationFunctionType
    ALU = mybir.AluOpType
    AX = mybir.AxisListType

    nc = bacc.Bacc(
        "TRN2",
        target_bir_lowering=False,
        debug=False,
        enable_asserts=False,
        num_devices=NCORES,
    )

    xallT_d = nc.declare_dram_parameter("xallT", [C, JPC, S], f16, isOutput=False)
    xg_d = nc.declare_dram_parameter("xg", [B_TOTAL, JPC * C], f16, isOutput=False)
    cp16_d = nc.declare_dram_parameter("cpack16", [98, 3936], f16, isOutput=False)
    cp32_d = nc.declare_dram_parameter("cpack32", [98, 216], f32, isOutput=False)
    out_d = nc.declare_dram_parameter(
        "out", [NJG, JW, C, IW, S, JW], f16, isOutput=True
    )

    with tile.TileContext(nc) as tc:
        with (
            tc.tile_pool(name="const", bufs=1) as cpool,
            tc.tile_pool(name="work", bufs=2) as work,
            tc.tile_pool(name="psum", bufs=8, space="PSUM") as pp,
        ):
            # ---- constants (packed tiles; views below) ----
            cp16 = cpool.tile([98, 3936], f16, name="cp16")
            cp32 = cpool.tile([98, 216], f32, name="cp32")
            aarr = cpool.tile([C, S, JPC], f16, name="aarr")  # [l, k, j]
            xarr = cpool.tile([C, B_TOTAL, JPC], f16, name="xarr")  # [l, i, j]
            xsc98 = cpool.tile([98, JPC, S], f16, name="xsc98")
            qkT = cpool.tile([DK, 2, NH, JPC, S], f16, name="qkT")

            wq_sb = cp16[:, 0:768].rearrange("p (h d) -> p h d", h=NH)
            wk_sb = cp16[:, 768:1536].rearrange("p (h d) -> p h d", h=NH)
            wv_sb = cp16[0:97, 1536:2304]
            wo_sb = cp16[0:DK, 2304:3072].rearrange("p (h l) -> p h l", h=NH)
            maskb_sb = cp16[0:S, 3072:3168]
            ones_sb = cp16[0:S, 3168:3264]
            iden4_sb = cp16[0:C, 3264:3648]
            iden_sb = cp16[0:C, 3648:3744]
            ones98_sb = cp16[0:C, 3744:3842]
            gvec_sb = cp32[0:C, 0:12]
            bb_sb = cp32[0:C, 12:24]
            gamma_rep = cp32[:, 24:120]
            beta_rep = cp32[:, 120:216]

            nc.sync.dma_start(out=cp16, in_=cp16_d[:])
            nc.sync.dma_start(out=cp32, in_=cp32_d[:])
            # rows 96-97 = 1.0 once; the loop overwrites row 96 with the
            # shift row each iteration (engines need 32-aligned start
            # partitions, so the two rows are set together)
            nc.vector.memset(xsc98[96:98, :, :], 1.0)

            inv_na = 1.0 / (C * C // G)  # 1/1152 per (batch, group)
            rc = RECIP_APPROX_FAST_CONSTS

            loop_cm = (
                tc.For_i(0, loop_n, 1, staggered_reset=True)
                if loop_n > 1
                else contextlib.nullcontext()
            )
            loop_cm.__enter__()

            def newton_rsqrt(veps, tag, iters=2):
                """rstd = rsqrt(veps), all-DVE (quake seed + Newton steps)
                so ACT only ever needs the Exp table set."""
                shp = list(veps.shape)
                iv = veps.bitcast(i32)
                ineg = work.tile(shp, i32, tag="sti", bufs=8, name="ineg" + tag)
                nc.vector.tensor_scalar_mul(ineg, iv, -1)
                nc.vector.tensor_scalar(ineg, ineg, 1, None, op0=ALU.arith_shift_right)
                nc.vector.tensor_scalar(ineg, ineg, 0x5F3759DF, None, op0=ALU.add)
                y = ineg.bitcast(f32)
                t1 = work.tile(shp, f32, tag="st", bufs=8, name="t1" + tag)
                for _ in range(iters):
                    nc.vector.tensor_mul(t1, y, y)
                    nc.vector.tensor_mul(t1, t1, veps)
                    nc.vector.tensor_scalar(t1, t1, -0.5, 1.5, op0=ALU.mult, op1=ALU.add)
                    nc.vector.tensor_mul(y, y, t1)
                return y

            # ===== phase 2 prologue: groupnorm stats over xallT, xsc98 ====
            xallT_sb = cpool.tile([C, JPC, S], f16, name="xallT_sb")
            if "2" in phases:
                nc.sync.dma_start(out=xallT_sb, in_=xallT_d[:])
                x2t = work.tile([C, JPC, S], f16, tag="x2t", name="x2t")
                nc.scalar.activation(out=x2t, in_=xallT_sb, func=AF.Square)
                s1v = work.tile([98, JPC, G], f32, tag="stv", bufs=8, name="s1v")
                s2v = work.tile([98, JPC, G], f32, tag="stv", bufs=8, name="s2v")
                for g3 in range(3):
                    bs = slice(4 * g3, 4 * (g3 + 1))
                    ps1 = pp.tile([98, 512], f32, tag="pss", bufs=1, name="ps_s1")
                    nc.tensor.matmul(
                        ps1[:, 0:384],
                        lhsT=ones98_sb,
                        rhs=xallT_sb[:, bs, :].rearrange("p b s -> p (b s)"),
                        start=True,
                        stop=True,
                    )
                    nc.vector.tensor_reduce(
                        out=s1v[:, bs, :].unsqueeze(3),
                        in_=ps1[:, 0:384].rearrange("p (b g s) -> p b g s", b=4, g=G),
                        axis=AX.X,
                        op=ALU.add,
                    )
                    ps2 = pp.tile([98, 512], f32, tag="pss", bufs=1, name="ps_s2")
                    nc.tensor.matmul(
                        ps2[:, 0:384],
                        lhsT=ones98_sb,
                        rhs=x2t[:, bs, :].rearrange("p b s -> p (b s)"),
                        start=True,
                        stop=True,
                    )
                    nc.vector.tensor_reduce(
                        out=s2v[:, bs, :].unsqueeze(3),
                        in_=ps2[:, 0:384].rearrange("p (b g s) -> p b g s", b=4, g=G),
                        axis=AX.X,
                        op=ALU.add,
                    )
                mu = work.tile([98, JPC, G], f32, tag="stv", bufs=8, name="mu")
                ex2 = work.tile([98, JPC, G], f32, tag="stv", bufs=8, name="ex2")
                nc.vector.tensor_scalar_mul(mu, s1v, inv_na)
                nc.vector.tensor_scalar_mul(ex2, s2v, inv_na)
                musq = work.tile([98, JPC, G], f32, tag="stv", bufs=8, name="musq")
                nc.vector.tensor_mul(musq, mu, mu)
                veps = work.tile([98, JPC, G], f32, tag="stv", bufs=8, name="veps")
                nc.vector.scalar_tensor_tensor(
                    veps, ex2, EPS, musq, op0=ALU.add, op1=ALU.subtract
                )
                y = newton_rsqrt(veps, "a", iters=2)
                # scale2[p, b, s] = rstd[b, g(s)] * gamma[s]; f16 so the
                # xsc multiply below runs in DVE 2x packed mode
                scale2 = work.tile([98, JPC, S], f16, tag="sc2", name="scale2")
                nc.vector.tensor_tensor(
                    scale2.rearrange("p b (g s) -> p b g s", g=G),
                    y.unsqueeze(3).to_broadcast((98, JPC, G, 12)),
                    gamma_rep.rearrange("p (g s) -> p g s", g=G)
                    .unsqueeze(1)
                    .to_broadcast((98, JPC, G, 12)),
                    ALU.mult,
                )
                # shift2 = beta - mu*scale2 (row 96 feeds the matmul
                # shift-row; rows 0-95 are computed but unused)
                msc = work.tile([98, JPC, S], f16, tag="msc", name="msc")
                nc.vector.tensor_tensor(
                    msc.rearrange("p b (g s) -> p b g s", g=G),
                    mu.unsqueeze(3).to_broadcast((98, JPC, G, 12)),
                    scale2.rearrange("p b (g s) -> p b g s", g=G),
                    ALU.mult,
                )
                shift2 = work.tile([98, JPC, S], f16, tag="sh2", name="shift2")
                nc.vector.tensor_tensor(
                    shift2,
                    beta_rep.unsqueeze(1).to_broadcast((98, JPC, S)),
                    msc,
                    ALU.subtract,
                )
                nc.vector.tensor_mul(xsc98[0:96, :, :], xallT_sb, scale2[0:96, :, :])
                nc.vector.tensor_copy(
                    out=xsc98[96:97, :, :], in_=shift2[96:97, :, :]
                )

            # ===== phase 1: xarr (xn of ALL 96 batches at this core's
            # 12 j-rows, transposed to [l, i, j]) =========================
            xn_s = cpool.tile([B_TOTAL, JPC, C], f16, name="xn_s")
            if "1" in phases:
                xg_sb = work.tile([B_TOTAL, JPC * C], f16, tag="xg", name="xg_sb")
                nc.sync.dma_start(out=xg_sb, in_=xg_d[:])
                sq = work.tile([B_TOTAL, JPC * C], f16, tag="sq", name="sq")
                nc.scalar.activation(out=sq, in_=xg_sb, func=AF.Square)
                s1 = work.tile([C, 1], f32, tag="st", bufs=8, name="s1g")
                s2 = work.tile([C, 1], f32, tag="st", bufs=8, name="s2g")
                nc.vector.tensor_reduce(out=s1, in_=xg_sb, axis=AX.X, op=ALU.add)
                nc.vector.tensor_reduce(out=s2, in_=sq, axis=AX.X, op=ALU.add)
                mu_g = work.tile([C, 1], f32, tag="st", bufs=8, name="mu_g")
                ex2_g = work.tile([C, 1], f32, tag="st", bufs=8, name="ex2_g")
                nc.vector.tensor_scalar_mul(mu_g, s1, inv_na)
                nc.vector.tensor_scalar_mul(ex2_g, s2, inv_na)
                musq_g = work.tile([C, 1], f32, tag="st", bufs=8, name="musq_g")
                nc.vector.tensor_mul(musq_g, mu_g, mu_g)
                veps_g = work.tile([C, 1], f32, tag="st", bufs=8, name="veps_g")
                nc.vector.scalar_tensor_tensor(
                    veps_g, ex2_g, EPS, musq_g, op0=ALU.add, op1=ALU.subtract
                )
                rstd_g = newton_rsqrt(veps_g, "g")
                nc.vector.tensor_scalar(
                    xn_s.rearrange("p j l -> p (j l)"),
                    xg_sb,
                    mu_g,
                    rstd_g,
                    op0=ALU.subtract,
                    op1=ALU.mult,
                )

            def xarr_quad(g):
                if "1" not in phases:
                    return
                for j in range(JW * g, JW * (g + 1)):
                    ps_t = pp.tile(
                        [C, B_TOTAL], f16, tag="pstr", bufs=1, name="ps_tj"
                    )
                    nc.tensor.transpose(ps_t, xn_s[:, j, :], iden_sb)
                    nc.scalar.activation(
                        out=xarr[:, :, j],
                        in_=ps_t,
                        func=AF.Identity,
                        scale=gvec_sb[:, j : j + 1],
                        bias=bb_sb[:, j : j + 1],
                    )

            # ===== per-quad q/k (batched over the quad's 4 batches) and
            # v: emitted at each quad's top so quad 0's scores start as
            # early as possible and later quads' projections overlap
            # earlier quads' stages on ACT slack ==========================
            v_all = cpool.tile([S, JPC, NH, DK], f16, name="v_all")
            xsc_bs = xsc98.rearrange("p b s -> p (b s)")

            def qkv_quad(g):
                lo = 384 * g
                for h in range(NH):
                    tA = pp.tile([DK, 1024], f32, tag="ps", bufs=3, name="ps_qk")
                    for qi, wsb in ((0, wq_sb), (1, wk_sb)):
                        nc.tensor.matmul(
                            tA[:, 512 * qi : 512 * qi + 384],
                            lhsT=wsb[:, h, :],
                            rhs=xsc_bs[:, lo : lo + 384],
                            start=True,
                            stop=True,
                        )
                    nc.scalar.activation(
                        out=qkT[:, :, h, 4 * g : 4 * (g + 1), :],
                        in_=tA.rearrange("p (u x) -> p u x", u=2)[
                            :, :, 0:384
                        ].rearrange("p u (b s) -> p u b s", b=4),
                        func=AF.Copy,
                    )
                for b in range(4 * g, 4 * (g + 1)):
                    psv = pp.tile([S, 1024], f32, tag="ps", bufs=3, name="ps_v")
                    nc.tensor.matmul(
                        psv[:, 0:384],
                        lhsT=xsc98[0:97, b, :],
                        rhs=wv_sb[:, 0:384],
                        start=True,
                        stop=True,
                    )
                    nc.tensor.matmul(
                        psv[:, 512:896],
                        lhsT=xsc98[0:97, b, :],
                        rhs=wv_sb[:, 384:768],
                        start=True,
                        stop=True,
                    )
                    nc.scalar.activation(
                        out=v_all[:, b].rearrange("p (u h) d -> p u (h d)", u=2),
                        in_=psv.rearrange("p (u x) -> p u x", u=2)[:, :, 0:384],
                        func=AF.Copy,
                    )

            # ===== attention stages for the 12 local batches ==============
            st = {}

            def st_s(b):
                d = st[b] = {}
                pst = pp.tile([S, 1024], f32, tag="ps", bufs=3, name="ps_sc")
                for hh in range(2):
                    off = 512 * hh
                    nc.tensor.matmul(
                        pst[:, off : off + 384],
                        lhsT=maskb_sb,
                        rhs=iden4_sb,
                        start=True,
                        stop=False,
                    )
                    for hl in range(4):
                        h = 4 * hh + hl
                        nc.tensor.matmul(
                            pst[:, off + hl * S : off + (hl + 1) * S],
                            lhsT=qkT[:, 1, h, b, :],
                            rhs=qkT[:, 0, h, b, :],
                            start=False,
                            stop=(hl == 3),
                        )
                expT = work.tile([S, NH, S], f16, tag="expT", bufs=4, name="expT")
                nc.scalar.activation(
                    out=expT.rearrange("p (u h) s -> p u (h s)", u=2),
                    in_=pst.rearrange("p (u x) -> p u x", u=2)[:, :, 0:384],
                    func=AF.Exp,
                )
                d["expT"] = expT

            def st_d(b):
                d = st[b]
                expT = d["expT"]
                psd = pp.tile([S, 1024], f32, tag="ps", bufs=3, name="ps_den")
                for hh in range(2):
                    nc.tensor.matmul(
                        psd[:, 512 * hh : 512 * hh + 384],
                        lhsT=ones_sb,
                        rhs=expT[:, 4 * hh : 4 * (hh + 1), :].rearrange(
                            "p h s -> p (h s)"
                        ),
                        start=True,
                        stop=True,
                    )
                recip = work.tile([S, NH, S], f16, tag="recip", bufs=4, name="recip")
                with nc.allow_low_precision(reason="softmax recip f16; 2e-2 gate"):
                    for hh in range(2):
                        nc.vector._custom_dve(
                            RECIPROCAL_APPROX_FAST,
                            out=recip[:, 4 * hh : 4 * (hh + 1), :].rearrange(
                                "p h s -> p (h s)"
                            ),
                            in0=psd[:, 512 * hh : 512 * hh + 384],
                            s0=rc["s0"],
                            s1=rc["s1"],
                            imm2=rc["imm2"],
                        )
                d["recip"] = recip

            def st_o(b):
                d = st[b]
                pso = pp.tile([DK, 1024], f32, tag="ps", bufs=3, name="ps_o")
                for hh in range(2):
                    for hl in range(4):
                        h = 4 * hh + hl
                        nc.tensor.matmul(
                            pso[:, 512 * hh + hl * S : 512 * hh + (hl + 1) * S],
                            lhsT=v_all[:, b, h, :],
                            rhs=d["expT"][:, h, :],
                            start=True,
                            stop=True,
                        )
                oraw = work.tile([DK, NH, S], f16, tag="oraw", bufs=4, name="oraw")
                nc.scalar.activation(
                    out=oraw.rearrange("p (u h) s -> p u (h s)", u=2),
                    in_=pso.rearrange("p (u x) -> p u x", u=2)[:, :, 0:384],
                    func=AF.Copy,
                )
                ocatT = work.tile([DK, NH, S], f16, tag="ocatT", bufs=4, name="ocatT")
                with nc.allow_low_precision(reason="attn weights f16; 2e-2 gate"):
                    nc.gpsimd.tensor_tensor(ocatT, oraw, d["recip"], ALU.mult)
                d["ocatT"] = ocatT

            def st_w(b):
                d = st.pop(b)
                pswt = pp.tile([C, 1024], f32, tag="ps", bufs=3, name="ps_w")
                psw = pswt[:, 0:S]
                for h in range(NH):
                    nc.tensor.matmul(
                        psw,
                        lhsT=wo_sb[:, h, :],
                        rhs=d["ocatT"][:, h, :],
                        start=(h == 0),
                        stop=(h == NH - 1),
                    )
                nc.scalar.activation(out=aarr[:, :, b], in_=psw, func=AF.Copy)

            # ===== assembly quad g: res[l, i24, k, j4] = A + X ===========
            dma_eng = nc.scalar if dma_ring == "scalar" else nc.sync

            def asm_quad(g):
                if "3" not in phases and "4" not in phases:
                    return
                js = slice(JW * g, JW * (g + 1))
                for ic in range(JW):
                    res = work.tile(
                        [C, IW, S, JW], f16, tag="res", bufs=3, name="res"
                    )
                    nc.vector.tensor_tensor(
                        res,
                        aarr[:, :, js].unsqueeze(1).to_broadcast((C, IW, S, JW)),
                        xarr[:, ic * IW : (ic + 1) * IW, js].unsqueeze(2).to_broadcast(
                            (C, IW, S, JW)
                        ),
                        ALU.add,
                    )
                    if "4" not in phases:
                        dma_eng.dma_start(out=out_d[g, ic], in_=res)

            # ===== schedule: quads of 4 batches, stage-interleaved; the
            # assembly of quad g-1 is emitted inside quad g's stages so
            # DVE chews on it while PE/ACT run quad g ====================
            if "2" in phases:
                for g in range(NJG):
                    qkv_quad(g)
                    xarr_quad(g)
                    bs = [4 * g + i for i in range(4)]
                    for fn in (st_s, st_d, st_o, st_w):
                        for b in bs:
                            fn(b)
                    asm_quad(g)
            elif "1" in phases or "3" in phases or "4" in phases:
                # assembly/DMA timing variants without attention
                nc.vector.memset(aarr[:], 0.0)
                if "1" not in phases and ("3" in phases or "4" in phases):
                    nc.vector.memset(xarr[:], 0.0)
                for g in range(NJG):
                    xarr_quad(g)
                    asm_quad(g)

            # DMA probe "z": out-DMA only, from one dummy buffer
            if "z" in phases:
                dm = cpool.tile([C, IW, S, JW], f16, name="dummy_res")
                nc.vector.memset(dm[:], 0.25)
                for g in range(NJG):
                    for ic in range(JW):
                        dma_eng.dma_start(out=out_d[g, ic], in_=dm)

            loop_cm.__exit__(None, None, None)

    nc.compile()
    return nc


def _get_program():
    global _PROG
    if _PROG is None:
        _PROG = _build_program()
    return _PROG


def _host_inputs(x, Wq, bq, Wk, bk, Wv, bv, Wo, bo, gamma, beta):
    f32 = np.float32
    f16 = np.float16
    x = np.asarray(x, f32)
    Wq = np.asarray(Wq, f32)
    bq = np.asarray(bq, f32)
    Wk = np.asarray(Wk, f32)
    bk = np.asarray(bk, f32)
    Wv = np.asarray(Wv, f32)
    bv = np.asarray(bv, f32)
    Wo = np.asarray(Wo, f32)
    bo = np.asarray(bo, f32)
    gamma = np.asarray(gamma, f32)
    beta = np.asarray(beta, f32)

    sc = f32(1.0 / np.sqrt(DK))
    bo_eff = (bv.astype(np.float64) @ Wo.astype(np.float64) + bo).astype(f32)

    cp16 = np.zeros((98, 3936), f16)
    cp16[0:96, 0:768] = (Wq * sc).astype(f16)
    cp16[96, 0:768] = (Wq.sum(axis=0) * sc).astype(f16)
    cp16[97, 0:768] = (bq * sc).astype(f16)
    cp16[0:96, 768:1536] = Wk.astype(f16)
    cp16[96, 768:1536] = Wk.sum(axis=0).astype(f16)
    cp16[97, 768:1536] = bk.astype(f16)
    cp16[0:96, 1536:2304] = Wv.astype(f16)
    cp16[96, 1536:2304] = Wv.sum(axis=0).astype(f16)
    cp16[0:96, 2304:3072] = (
        Wo.reshape(NH, DK, C).transpose(1, 0, 2).reshape(DK, 768).astype(f16)
    )
    # maskbT[q, t] = -30 where t > q (causal), added to scoresT in-psum
    cp16[0:S, 3072:3168] = np.triu(np.full((S, S), -30.0, f16), 1)
    cp16[0:S, 3168:3264] = np.ones((S, S), f16)
    cp16[0:C, 3264:3648] = np.broadcast_to(
        np.eye(C, dtype=f16)[:, None, :], (C, 4, S)
    ).reshape(C, 384)
    cp16[0:C, 3648:3744] = np.eye(C, dtype=f16)
    cp16[0:C, 3744:3842] = np.ones((C, 98), f16)

    com = {"cpack16": cp16}
    x_r = np.ascontiguousarray(x.reshape(B_TOTAL, C, C))
    in_maps = []
    for c in range(NCORES):
        J = slice(c * JPC, (c + 1) * JPC)
        m = dict(com)
        # xallT[c, b, s] = x[local b, s(chan), c(w)] transposed
        m["xallT"] = np.ascontiguousarray(
            x_r[J].transpose(2, 0, 1).astype(f16)
        )
        m["xg"] = (
            np.ascontiguousarray(x_r[:, J, :])
            .reshape(B_TOTAL, JPC * C)
            .astype(f16)
        )
        cp32 = np.zeros((98, 216), f32)
        cp32[0:C, 0:12] = np.broadcast_to(gamma[J][None, :], (C, JPC))
        cp32[0:C, 12:24] = beta[J][None, :] + bo_eff[:, None]
        cp32[:, 24:120] = np.broadcast_to(gamma[None, :], (98, C))
        cp32[:, 120:216] = np.broadcast_to(beta[None, :], (98, C))
        m["cpack32"] = cp32
        in_maps.append(m)
    return in_maps


def _assemble(parts):
    """parts[c]: (NJG, JW, C, IW, S, JW) f16 [jq, ic, l, i24, k, j4]
    -> (B, C, S, C) f32."""
    cols = []
    for a in parts:
        a = np.asarray(a).astype(np.float32).reshape(NJG, JW, C, IW, S, JW)
        # (jq, ic, l, i24, k, j4) -> (ic, i24, jq, j4, k, l)
        a = a.transpose(1, 3, 0, 5, 4, 2).reshape(B_TOTAL, JPC, S, C)
        cols.append(a)
    return np.concatenate(cols, axis=1)


def _run(inputs, trace=False):
    from concourse.bass_utils import run_bass_kernel_spmd

    nc = _get_program()
    in_maps = _host_inputs(**inputs)
    res = run_bass_kernel_spmd(
        nc, in_maps, core_ids=list(range(NCORES)), trace=trace
    )
    out = _assemble([r["out"] for r in res.results])
    return out, res


def kernel(**inputs) -> np.ndarray:
    out, _ = _run(inputs, trace=False)
    return out
